# revision 1
# baseline (speedup 1.0000x reference)
"""ClassAttention kernel for 8x TRN2 NeuronCores.

Reference computation (per batch element):
    qkv = x @ qkv_w.T + qkv_b                      # [N, 3C]
    q, k, v = split(qkv)                           # heads H=12, D=64
    s = softmax((q_cls . k) / sqrt(D))             # class-token query only
    cls = (s @ v) @ proj_w.T + proj_b              # [1, C]
    out = concat([cls, x[1:]])                     # rows 1..N pass through

Only the class token row changes, so the device kernel computes just the
[B, C] cls output; rows 1..N are passed through on the host.

Sharding: data-parallel over batch, 8 batches per core, no collectives.
Compute dtype bf16 (fp32 PSUM accumulation), fp32 output.

Key algebraic restructuring (exploits the single class-token query):
  - scores fold the k-projection into a per-batch effective query in
    x-space:  s[b,h,n] = sum_c Wt[c, b*12+h] x[b,n,c]  with
    Wt = wk.T @ blockdiag(q) computed ONCE for all batches (768x96).
    No k vector is ever materialized.
  - the v-projection commutes with the attention average:
    o = p.T @ (x @ wv.T) = (p.T @ x) @ wv.T, so we compute the
    attention-weighted average of x (Z = p.T @ x, 12x768 per batch)
    and project it through wv once per batch. No v vector is ever
    materialized. This collapses the dominant [577x768]x[768x768]
    per-batch projection into [12x768]x[768x768].
  - k bias shifts every score of a head by the same constant ->
    cancels in softmax; dropped.
  - v bias contributes sum_n(p)=1 times vb to the attention output ->
    folds into the proj bias on the host: pb_eff = proj_b + vb @ proj_w.T.
  - softmax skips the max-shift (scores are O(1): q.k/sqrt(D) with
    unit-variance inputs, so exp() cannot overflow) and the 1/sum
    normalization is folded into the Z psum evacuation.

Per-core dataflow (b = 0..8 batches, C=768 in 6 chunks of 128):
  xT[c, b, n], x2[b, n, c]   both layouts of x, host-cast bf16
  qc[b, o]    = wqT.T @ xcls + qb    (wq,qb pre-scaled by 1/8 on host)
  Qblk[o, bh] = block-diagonal q     (PE transpose + aligned copies)
  Wt[c, bh]   = wk2.T @ Qblk         (36 matmuls, once)
  s_b[h, n]   = Wt_b.T @ xT_b
  p_b         = exp(s)                (unnormalized, bf16)
  Z_b[h, c]   = pT_b.T @ x2_b / sum   (attention average of x)
  o           = ZT_all.T @ wvT        (M-packed: 4 batches per 128-col
                                       group at 32-aligned psum rows)
  oT_vec[o,b] = diag-extract          (PE transpose + aligned copies)
  cls[b, :]   = oT.T @ wpT + pb_eff

The loop is software-pipelined: scores/softmax run two batches ahead,
diag-extraction trails four batches behind, and the o-projection runs
once per 4-batch group, so PE work always covers the DVE/ACT
round-trips. The DMA stream (x in both layouts + weights, ~19 MB) is
the modeled bottleneck; DMAs are emitted in consumption order.
"""

import functools

import numpy as np
import ml_dtypes

import concourse.bass as bass
import concourse.tile as tile
from concourse import bacc, mybir, masks
from concourse import bass_utils

BF16 = mybir.dt.bfloat16
F32 = mybir.dt.float32
NPBF16 = ml_dtypes.bfloat16

B, N, C = 64, 577, 768
H, D = 12, 64
NCORES = 8
BPC = B // NCORES          # 8 batches per core
CT = C // 128              # 6 chunks of the feature dim
SCALE = D ** -0.5          # folded into wq/qb on the host

# token splits: matmul free dim (<=512 fp32 psum bank), K-chunks (<=128)
N_HALVES = [(0, 289), (289, 288)]
C_HALVES = [(0, 512), (512, 256)]
T_TILES = [(0, 128), (128, 128), (256, 128), (384, 128), (512, 65)]
NTT = len(T_TILES)


def build_module():
    nc = bacc.Bacc("TRN2", target_bir_lowering=False, debug=False)

    xT_d = nc.dram_tensor("xT", [C, BPC, N], BF16, kind="ExternalInput")
    x2_d = nc.dram_tensor("x2", [BPC, N, C], BF16, kind="ExternalInput")
    wk2_d = nc.dram_tensor("wk2", [C, C], BF16, kind="ExternalInput")  # [o, c]
    wv_d = nc.dram_tensor("wv", [C, C], BF16, kind="ExternalInput")    # [c, o]
    wq_d = nc.dram_tensor("wq", [C, C], BF16, kind="ExternalInput")    # [c, o]
    wp_d = nc.dram_tensor("wp", [C, C], BF16, kind="ExternalInput")    # [c, o]
    xcls_d = nc.dram_tensor("xcls", [C, BPC], BF16, kind="ExternalInput")
    wtqb_d = nc.dram_tensor("wtqb", [C, BPC * H], BF16, kind="ExternalInput")
    pb_d = nc.dram_tensor("pb", [BPC, C], F32, kind="ExternalInput")
    cls_d = nc.dram_tensor("cls", [BPC, C], F32, kind="ExternalOutput")

    AF = mybir.ActivationFunctionType
    AX = mybir.AxisListType

    with tile.TileContext(nc) as tc:
        with (
            tc.tile_pool(name="const", bufs=1) as const,
            tc.tile_pool(name="xp", bufs=4) as xp,
            tc.tile_pool(name="x2p", bufs=5) as x2p,
            tc.tile_pool(name="sm", bufs=3) as sm,
            tc.tile_pool(name="ps", bufs=2, space="PSUM") as ps,
        ):
            # ---- DMAs, in the order the pipeline consumes them ----
            # (the cost model serializes dma_starts on one channel)
            xcls = const.tile([128, CT, BPC], BF16, tag="xcls")
            nc.sync.dma_start(
                xcls[:], xcls_d.ap().rearrange("(a p) b -> p a b", p=128))
            wq = const.tile([128, CT, C], BF16, tag="wq")
            wqr = wq_d.ap().rearrange("(a p) o -> p a o", p=128)
            for ci in range(CT):
                nc.sync.dma_start(wq[:, ci, :], wqr[:, ci, :])
            wk2 = const.tile([128, CT, C], BF16, tag="wk2")
            wk2r = wk2_d.ap().rearrange("(a p) o -> p a o", p=128)
            for ci in range(CT):
                nc.sync.dma_start(wk2[:, ci, :], wk2r[:, ci, :])
            wtqbr = const.tile([128, CT, BPC * H], BF16, tag="wtqbr")
            nc.sync.dma_start(
                wtqbr[:], wtqb_d.ap().rearrange("(a p) o -> p a o", p=128))

            xbs = {}

            def load_xb(b):
                xb = xp.tile([128, CT, N], BF16, tag="xb")
                nc.sync.dma_start(
                    xb[:],
                    xT_d.ap()[:, b, :].rearrange("(a p) t -> p a t", p=128))
                xbs[b] = xb

            x2s = {}

            def load_x2(b):
                x2 = x2p.tile([128, NTT, C], BF16, tag="x2")
                for ti, (to, tw) in enumerate(T_TILES):
                    nc.sync.dma_start(
                        x2[:tw, ti, :], x2_d.ap()[b, to:to + tw, :])
                x2s[b] = x2

            load_xb(0)
            load_x2(0)
            wv = const.tile([128, CT, C], BF16, tag="wv")
            wvr = wv_d.ap().rearrange("(a p) o -> p a o", p=128)
            nc.sync.dma_start(wv[:, :, 0:512], wvr[:, :, 0:512])
            load_xb(1)
            nc.sync.dma_start(wv[:, :, 512:768], wvr[:, :, 512:768])
            load_x2(1)
            load_xb(2)
            load_x2(2)
            pbr = const.tile([BPC, C], F32, tag="pbr")
            wp = const.tile([128, CT, C], BF16, tag="wp")

            identb = const.tile([12, 12], BF16, tag="identb")
            masks.make_identity(nc, identb[:])
            identf = const.tile([BPC, BPC], F32, tag="identf")
            masks.make_identity(nc, identf[:])

            Qblk = const.tile([128, CT, BPC * H], BF16, tag="Qblk")
            nc.vector.memset(Qblk[:], 0.0)
            Wt = const.tile([128, CT, BPC * H], BF16, tag="Wt")
            oT_vec = const.tile([128, CT, BPC], BF16, tag="oT_vec")
            q_sb = const.tile([BPC, C], F32, tag="q_sb")
            # ZT for 4 batches per 128-col group, 32-aligned (zero padding)
            ZT_all = const.tile([128, CT, 2, 128], BF16, tag="ZT_all")
            nc.vector.memset(ZT_all[:], 0.0)
            o_sb = const.tile([128, 2, C], BF16, tag="o_sb")
            # identity replicated at partition bases 0/32/64/96 for the
            # 32-aligned diag-extraction transposes
            ident4 = const.tile([128, H], BF16, tag="ident4")
            for _j in range(4):
                masks.make_identity(nc, ident4[32 * _j:32 * _j + H, :])

            # ---- q for all 8 batches, Qblk, Wt ----
            for ho, hw in C_HALVES:
                pq = ps.tile([BPC, 512], F32, tag="pav")
                for ci in range(CT):
                    nc.tensor.matmul(
                        pq[:, :hw], xcls[:, ci, :], wq[:, ci, ho:ho + hw],
                        start=(ci == 0), stop=(ci == CT - 1))
                nc.vector.tensor_copy(q_sb[:, ho:ho + hw], pq[:, :hw])

            QblkV = Qblk[:].rearrange("p a (b h) -> p a b h", h=H)
            for ci in range(CT):
                pqt = ps.tile([128, BPC], F32, tag="pav")
                nc.tensor.transpose(
                    pqt[:, :], q_sb[:, ci * 128:(ci + 1) * 128],
                    identf[:BPC, :BPC])
                for j in range(2):
                    h = 2 * ci + j
                    nc.vector.tensor_copy(
                        QblkV[j * 64:(j + 1) * 64, ci, :, h],
                        pqt[j * 64:(j + 1) * 64, :])

            for cj in range(CT):
                pw = ps.tile([128, BPC * H], F32, tag="pav")
                for oj in range(CT):
                    nc.tensor.matmul(
                        pw[:, :], wk2[:, oj, cj * 128:(cj + 1) * 128],
                        Qblk[:, oj, :],
                        start=(oj == 0), stop=(oj == CT - 1))
                nc.vector.tensor_add(
                    Wt[:, cj, :], pw[:, :], wtqbr[:, cj, :])

            # ---- per-batch software pipeline ----
            # Emission (= PE execution) order per slot k:
            #   A(k+2) scores | S(k+2) softmax (DVE/ACT) | X(k-1) extract |
            #   P(k) pT | Z(k) | T(k) ZT | O(k) attn-out
            # so every DVE/ACT round-trip is covered by PE work from a
            # neighboring slot. Psum evacuations are interleaved inside the
            # Z/O chains (half0 evacuates while half1's matmuls stream).
            st = {}

            def emit_A(b):
                xb = xbs.pop(b)
                pss = []
                for no, nw in N_HALVES:
                    s_ps = ps.tile([H, 512], F32, tag="pscz")
                    for ci in range(CT):
                        nc.tensor.matmul(
                            s_ps[:, :nw],
                            Wt[:, ci, b * H:(b + 1) * H],
                            xb[:, ci, no:no + nw],
                            start=(ci == 0), stop=(ci == CT - 1))
                    pss.append(s_ps)
                st[b] = {"pss": pss}

            def emit_S(b):
                s = st[b]
                pss = s["pss"]
                # scores are O(1) (q.k/8, unit-variance inputs): exp() is
                # safe without the max shift, which softmax cancels anyway
                e_bf = sm.tile([H, N], BF16, tag="e_bf")
                sums = []
                for i, (no, nw) in enumerate(N_HALVES):
                    acc = sm.tile([H, 1], F32, tag=f"acc{i}")
                    nc.scalar.activation(
                        e_bf[:, no:no + nw], pss[i][:, :nw], AF.Exp,
                        bias=0.0, scale=1.0, accum_out=acc[:])
                    sums.append(acc)
                ssum = sm.tile([H, 1], F32, tag="ssum")
                nc.vector.tensor_add(ssum[:], sums[0][:], sums[1][:])
                rden = sm.tile([H, 1], F32, tag="rden")
                nc.vector.reciprocal(rden[:], ssum[:])
                s["e_bf"], s["rden"] = e_bf, rden

            def emit_PZ(b):
                s = st[b]
                e_bf = s["e_bf"]
                pT = sm.tile([128, NTT, H], BF16, tag="pT")
                for ti, (to, tw) in enumerate(T_TILES):
                    ppt = ps.tile([128, H], BF16, tag="ptrb")
                    nc.tensor.transpose(
                        ppt[:tw, :], e_bf[:, to:to + tw], identb[:H, :H])
                    nc.vector.tensor_copy(pT[:tw, ti, :], ppt[:tw, :])

                x2 = x2s.pop(b)
                z_sb = sm.tile([H, C], BF16, tag="z_sb")
                for ho, hw in C_HALVES:
                    pz = ps.tile([H, 512], F32, tag="pscz")
                    for ti, (to, tw) in enumerate(T_TILES):
                        nc.tensor.matmul(
                            pz[:, :hw],
                            pT[:tw, ti, :],
                            x2[:tw, ti, ho:ho + hw],
                            start=(ti == 0), stop=(ti == NTT - 1))
                    nc.vector.tensor_scalar_mul(
                        z_sb[:, ho:ho + hw], pz[:, :hw], s["rden"][:])
                s["z_sb"] = z_sb

            def emit_T(b):
                # ZT for batch b into its 32-aligned column group of ZT_all
                s = st.pop(b)
                z_sb = s["z_sb"]
                g, j = b // 4, b % 4
                for ci in range(CT):
                    pzt = ps.tile([128, H], BF16, tag="ptrb")
                    nc.tensor.transpose(
                        pzt[:, :], z_sb[:, ci * 128:(ci + 1) * 128],
                        identb[:H, :H])
                    nc.vector.tensor_copy(
                        ZT_all[:, ci, g, 32 * j:32 * j + H], pzt[:, :])

            def emit_O(g):
                # o rows for 4 batches at once: psum rows 32j..32j+12 = batch
                # 4g+j  (M-packed; zero columns of ZT_all give zero rows)
                for ho, hw in C_HALVES:
                    po = ps.tile([128, 512], F32, tag="pav")
                    for ci in range(CT):
                        nc.tensor.matmul(
                            po[:, :hw],
                            ZT_all[:, ci, g, :],
                            wv[:, ci, ho:ho + hw],
                            start=(ci == 0), stop=(ci == CT - 1))
                    nc.scalar.copy(o_sb[:, g, ho:ho + hw], po[:, :hw])

            def emit_X(b):
                # extract diagonal blocks of batch b, transposed: oT_vec[o, b]
                g, j = b // 4, b % 4
                for ci in range(CT):
                    pot = ps.tile([128, H], BF16,
                                  tag="ptrb" if ci % 2 == 0 else "pav")
                    nc.tensor.transpose(
                        pot[:, :],
                        o_sb[32 * j:32 * j + H, g, ci * 128:(ci + 1) * 128],
                        ident4[32 * j:32 * j + H, :],
                        tile_position=(32 * j, 0))
                    for jj in range(2):
                        h = 2 * ci + jj
                        nc.vector.tensor_copy(
                            oT_vec[jj * 64:(jj + 1) * 64, ci, b:b + 1],
                            pot[jj * 64:(jj + 1) * 64, h:h + 1])

            emit_A(0)
            emit_S(0)
            emit_A(1)
            emit_S(1)
            for k in range(BPC):
                if k + 3 < BPC:
                    load_xb(k + 3)
                    load_x2(k + 3)
                if k == 5:
                    nc.sync.dma_start(pbr[:], pb_d.ap())
                    nc.sync.dma_start(
                        wp[:], wp_d.ap().rearrange("(a p) o -> p a o", p=128))
                if k + 2 < BPC:
                    emit_A(k + 2)
                    emit_S(k + 2)
                emit_PZ(k)
                emit_T(k)
                if k == 3:
                    emit_O(0)
                if k >= 4:
                    emit_X(k - 4)
            emit_O(1)
            for b in range(4, BPC):
                emit_X(b)

            # ---- proj for all 8 batches ----
            cls_sb = const.tile([BPC, C], F32, tag="cls_sb")
            for ho, hw in C_HALVES:
                pc = ps.tile([BPC, 512], F32, tag="pav")
                for ci in range(CT):
                    nc.tensor.matmul(
                        pc[:, :hw], oT_vec[:, ci, :], wp[:, ci, ho:ho + hw],
                        start=(ci == 0), stop=(ci == CT - 1))
                nc.vector.tensor_add(
                    cls_sb[:, ho:ho + hw], pc[:, :hw], pbr[:, ho:ho + hw])
            nc.sync.dma_start(cls_d.ap(), cls_sb[:])

    nc.compile()
    return nc


@functools.lru_cache(maxsize=1)
def _module():
    return build_module()


def make_in_maps(x, qkv_w, qkv_b, proj_w, proj_b):
    x = np.asarray(x, dtype=np.float32)
    qkv_w = np.asarray(qkv_w, dtype=np.float32)
    qkv_b = np.asarray(qkv_b, dtype=np.float32)
    proj_w = np.asarray(proj_w, dtype=np.float32)
    proj_b = np.asarray(proj_b, dtype=np.float32)

    wk2 = np.ascontiguousarray(qkv_w[C:2 * C]).astype(NPBF16)       # [o, c]
    wv = np.ascontiguousarray(qkv_w[2 * C:].T).astype(NPBF16)       # [c, o]
    wq = np.ascontiguousarray(qkv_w[:C].T * SCALE).astype(NPBF16)   # [c, o]
    wp = np.ascontiguousarray(proj_w.T).astype(NPBF16)              # [c, o]
    # q-bias folds into Wt: wtqb[c, h] = wk_block_h[:, c] . qb_block_h
    qbs = qkv_b[:C] * SCALE
    wtqb1 = np.stack(
        [qkv_w[C + 64 * h:C + 64 * (h + 1)].T @ qbs[64 * h:64 * (h + 1)]
         for h in range(H)], axis=1)                                # [C, H]
    wtqb = np.tile(wtqb1, (1, BPC)).astype(NPBF16)                  # [C, 96]
    # v bias contributes exactly (vb @ proj_w.T) to cls; fold into proj bias
    pb_eff = proj_b + qkv_b[2 * C:] @ proj_w.T
    pb = np.tile(pb_eff, (BPC, 1)).astype(np.float32)               # [8, C]

    in_maps = []
    for i in range(NCORES):
        xs = x[i * BPC:(i + 1) * BPC]                               # [8, N, C]
        x2 = xs.astype(NPBF16)                                      # [8, N, C]
        xT = np.ascontiguousarray(xs.transpose(2, 0, 1)).astype(NPBF16)
        xcls = np.ascontiguousarray(xs[:, 0, :].T).astype(NPBF16)   # [C, 8]
        in_maps.append({
            "xT": xT, "x2": x2, "wk2": wk2, "wv": wv, "wq": wq, "wp": wp,
            "xcls": xcls, "wtqb": wtqb, "pb": pb,
        })
    return in_maps


def kernel(x, qkv_w, qkv_b, proj_w, proj_b):
    nc = _module()
    in_maps = make_in_maps(x, qkv_w, qkv_b, proj_w, proj_b)
    res = bass_utils.run_bass_kernel_spmd(
        nc, in_maps, core_ids=list(range(NCORES)))
    out = np.array(np.asarray(x), dtype=np.float32, copy=True)
    for i in range(NCORES):
        out[i * BPC:(i + 1) * BPC, 0, :] = res.results[i]["cls"]
    return out



# revision 10
# speedup vs baseline: 2.0242x; 2.0242x over previous
"""ClassAttention kernel for 8x TRN2 NeuronCores — fp8 DoubleRow rewrite.

Reference computation (per batch element):
    qkv = x @ qkv_w.T + qkv_b                      # [N, 3C]
    q, k, v = split(qkv)                           # heads H=12, D=64
    s = softmax((q_cls . k) / sqrt(D))             # class-token query only
    cls = (s @ v) @ proj_w.T + proj_b              # [1, C]
    out = concat([cls, x[1:]])                     # rows 1..N pass through

Only the class token row changes, so the device computes just the [B, C]
cls output (shipped transposed as clsT [C, B]); rows 1..N pass through on
the host.  Data-parallel over batch: 8 batches per core, no collectives.

Algebraic structure (inherited from the bf16 baseline):
  - k-projection folds into x-space:  s[b,h,n] = sum_c Wt[c,bh] x[b,n,c]
    with Wt = wk.T @ blockdiag(q) computed once on device; no k vector is
    materialized.  k-bias cancels in softmax; q-bias folds into Wt via a
    host-precomputed wtqb.
  - v-projection commutes with the attention average: the kernel averages
    x (ZT = x.T @ p) and projects through wv once; v-bias folds into the
    proj bias on the host.
  - softmax skips the max-shift (scores are O(1)); the 1/sum scaling is
    applied per (b,h) column during the ZT psum evacuation.

What is new vs the baseline (82.2us -> ~35us modeled):
  - fp8(e4m3) data path: x (both layouts), wv, wp, Wt, p=exp(s), ZT, oT
    are fp8; the score-weight path (wq, wk2, q, Qblk, Wt accumulation)
    stays bf16 because it dominates the error budget.  DoubleRow fp8
    matmuls (2 K-tiles per instruction, 0.5 cycles/row) carry all the
    heavy contractions.
  - every stage computes the TRANSPOSED output with a small moving free
    dim (qT, sT, ZT, oT, clsT), so there are ZERO data transposes and
    psum evacuations are few and wide ([128, .] copies, not [12, .]).
  - 21 large DMAs instead of 67 (HWDGE issue cost ~630ns each gated the
    baseline); x2 is read as [128, 5, 768] per batch from a 63-row-padded
    flat buffer so each batch is one descriptor-dense transfer.

Per-core dataflow (b = 0..8 batches, c in 6 chunks of 128):
  qT[o, b]        36 bf16 matmuls      (needs xcls, wq)
  Qblk[o, (b h)]  12 blockdiag copies  (DVE, psum -> bf16)
  Wt[c, (b h)]    36 bf16 matmuls + 6 adds (+wtqb, cast fp8)
  sT[n, (b h)]    120 DR matmuls       (needs all xT)
  pT = exp(sT-1)  2 Act ops, fp8       (bias cancels in the 1/sum)
  sums[1, (b h)]  3 ones-matmuls; rden = 1/sums (f32)
  rdenB[o, (b h)] 2 outer-product matmuls + copy
  ZT[c, (g j h)]  144 DR matmuls       (needs x2_b), x rden -> fp8
  oT[o', b]       72 DR matmuls        (diag blocks direct, needs wv)
  clsT[j, b]      36 DR matmuls + pbT add -> f32, DMA out per group
"""

import functools

import numpy as np
import ml_dtypes

import concourse.bass as bass
import concourse.tile as tile
from concourse import bacc, mybir
from concourse import bass_utils

BF16 = mybir.dt.bfloat16
F8 = mybir.dt.float8e4
F32 = mybir.dt.float32
NPBF16 = ml_dtypes.bfloat16
NPF8 = ml_dtypes.float8_e4m3
DR = mybir.MatmulPerfMode.DoubleRow

B, N, C = 64, 577, 768
H, D = 12, 64
NCORES = 8
BPC = B // NCORES          # 8 batches per core
CT = C // 128              # 6 chunks of the feature dim
NT = 5                     # token tiles of 128 (last holds 65)
NTAIL = N - 4 * 128        # 65
SCALE = D ** -0.5          # folded into wq on the host
X2PAD = 5 * 128 - N        # 63 rows of row padding after the last batch


def build_module():
    nc = bacc.Bacc("TRN2", target_bir_lowering=False, debug=False)

    xT_d = nc.dram_tensor("xT", [C, BPC, N], F8, kind="ExternalInput")
    x2_d = nc.dram_tensor("x2", [BPC * N + X2PAD, C], F8, kind="ExternalInput")
    wq_d = nc.dram_tensor("wq", [C, C], BF16, kind="ExternalInput")    # [c, o]
    wk2_d = nc.dram_tensor("wk2", [C, C], BF16, kind="ExternalInput")  # [o, c]
    wv_d = nc.dram_tensor("wv", [C, C], F8, kind="ExternalInput")      # [c, o]
    wp_d = nc.dram_tensor("wp", [C, C], F8, kind="ExternalInput")      # [c, o]
    xcls_d = nc.dram_tensor("xcls", [C, BPC], BF16, kind="ExternalInput")
    wtqb_d = nc.dram_tensor("wtqb", [C, BPC * H], BF16, kind="ExternalInput")
    pbT_d = nc.dram_tensor("pbT", [C, BPC], F32, kind="ExternalInput")
    clsT_d = nc.dram_tensor("clsT", [C, BPC], F32, kind="ExternalOutput")

    AF = mybir.ActivationFunctionType

    with tile.TileContext(nc) as tc:
        with (
            tc.tile_pool(name="sb", bufs=1) as sb,
            tc.tile_pool(name="psA", bufs=2, space="PSUM") as psA,
            tc.tile_pool(name="psW", bufs=2, space="PSUM") as psW,
            tc.tile_pool(name="psS", bufs=1, space="PSUM") as psS,
            tc.tile_pool(name="psR", bufs=1, space="PSUM") as psR,
            tc.tile_pool(name="psZ", bufs=2, space="PSUM") as psZ,
        ):
            # ---- DMAs, in consumption order (one channel, serialized) ----
            xcls = sb.tile([128, CT, BPC], BF16, tag="xcls")
            nc.sync.dma_start(
                xcls[:], xcls_d.ap().rearrange("(a p) b -> p a b", p=128))
            wq = sb.tile([128, CT, C], BF16, tag="wq")
            nc.sync.dma_start(
                wq[:], wq_d.ap().rearrange("(a p) o -> p a o", p=128))
            wk2 = sb.tile([128, CT, C], BF16, tag="wk2")
            nc.sync.dma_start(
                wk2[:], wk2_d.ap().rearrange("(a p) o -> p a o", p=128))
            wtqb = sb.tile([128, CT, BPC * H], BF16, tag="wtqb")
            nc.sync.dma_start(
                wtqb[:], wtqb_d.ap().rearrange("(a p) o -> p a o", p=128))
            # x in c-major layout, 2 batches per DMA
            xTp = []
            for i in range(BPC // 2):
                xt = sb.tile([128, CT, 2, N], F8, tag=f"xT{i}")
                nc.sync.dma_start(
                    xt[:].rearrange("p a b t -> p a (b t)"),
                    xT_d.ap()[:, 2 * i:2 * i + 2, :]
                    .rearrange("(a p) b t -> p a (b t)", p=128))
                xTp.append(xt)
            wv = sb.tile([128, CT, C], F8, tag="wv")
            nc.sync.dma_start(
                wv[:], wv_d.ap().rearrange("(a p) o -> p a o", p=128))
            wp = sb.tile([128, CT, C], F8, tag="wp")
            nc.sync.dma_start(
                wp[:], wp_d.ap().rearrange("(a p) o -> p a o", p=128))
            pbT = sb.tile([128, CT, BPC], F32, tag="pbT")
            nc.sync.dma_start(
                pbT[:], pbT_d.ap().rearrange("(a p) b -> p a b", p=128))
            # x in token-major layout, one overlapping [640, C] read per
            # batch (rows past token 577 belong to the next batch / the host
            # pad and are masked by exact-K tail matmuls)
            x2s = []
            for b in range(BPC):
                x2 = sb.tile([128, NT, C], F8, tag=f"x2{b}")
                nc.sync.dma_start(
                    x2[:],
                    x2_d.ap()[b * N:b * N + NT * 128, :]
                    .rearrange("(a p) c -> p a c", p=128))
                x2s.append(x2)

            # ---- small constants ----
            ones8 = sb.tile([128, 2, 1], F8, tag="ones8")
            nc.vector.memset(ones8[:], 1.0)
            negone = sb.tile([128, 1], F32, tag="negone")
            nc.vector.memset(negone[:], -1.0)
            onesf = sb.tile([1, 128], F32, tag="onesf")
            nc.vector.memset(onesf[:], 1.0)
            Qblk = sb.tile([128, CT, BPC * H], BF16, tag="Qblk")
            nc.vector.memset(Qblk[:], 0.0)

            Wt = sb.tile([128, CT, BPC * H], F8, tag="Wt")
            pT = sb.tile([128, NT, BPC, H], F8, tag="pT")
            rden = sb.tile([1, BPC * H], F32, tag="rden")
            rdenB = sb.tile([128, 2, 1, 48], F32, tag="rdenB")
            ZT = sb.tile([128, CT, 2, 48], F8, tag="ZT")
            oT = sb.tile([128, CT, BPC], F8, tag="oT")
            clsT_sb = sb.tile([128, CT, BPC], F32, tag="clsT_sb")

            # ---- qT[o, b]: 36 bf16 matmuls, out free dim 8 ----
            pq = psA.tile([128, CT, BPC], F32, tag="A")
            for oc in range(CT):
                for ck in range(CT):
                    nc.tensor.matmul(
                        pq[:, oc, :],
                        wq[:, ck, 128 * oc:128 * (oc + 1)],
                        xcls[:, ck, :],
                        start=(ck == 0), stop=(ck == CT - 1))

            # ---- Qblk[o, (b h)]: blockdiag scatter of qT (bf16) ----
            QblkV = Qblk[:].rearrange("p a (b h) -> p a b h", h=H)
            for oc in range(CT):
                for j in range(2):
                    h = 2 * oc + j
                    nc.vector.tensor_copy(
                        QblkV[64 * j:64 * (j + 1), oc, :, h],
                        pq[64 * j:64 * (j + 1), oc, :])

            # ---- Wt[c, (b h)] = wk2.T @ Qblk + wtqb, cast fp8 ----
            for cj in range(CT):
                pw = psW.tile([128, BPC * H], F32, tag="W")
                for ok in range(CT):
                    nc.tensor.matmul(
                        pw[:], wk2[:, ok, 128 * cj:128 * (cj + 1)],
                        Qblk[:, ok, :],
                        start=(ok == 0), stop=(ok == CT - 1))
                nc.vector.tensor_add(Wt[:, cj, :], pw[:], wtqb[:, cj, :])

            # ---- sT[n, (b h)] per batch: 15 DR matmuls over c ----
            ps_s = psS.tile([128, NT, BPC, H], F32, tag="S")
            for b in range(BPC):
                i, j = b // 2, b % 2
                for nt in range(NT):
                    w = 128 if nt < NT - 1 else NTAIL
                    off = 128 * nt
                    for t in range(3):
                        nc.tensor.matmul(
                            ps_s[:w, nt, b, :],
                            xTp[i][:, 2 * t:2 * t + 2, j, off:off + w],
                            Wt[:, 2 * t:2 * t + 2, H * b:H * (b + 1)],
                            start=(t == 0), stop=(t == 2), perf_mode=DR)

            # ---- pT = exp(sT - 1), fp8 (the -1 cancels in 1/sum and
            #      keeps e below the fp8e4 max) ----
            nc.scalar.activation(
                pT[:, 0:4, :, :], ps_s[:, 0:4, :, :], AF.Exp,
                bias=negone[:], scale=1.0)
            nc.scalar.activation(
                pT[:NTAIL, 4, :, :], ps_s[:NTAIL, 4, :, :], AF.Exp,
                bias=negone[:NTAIL, :], scale=1.0)

            # ---- sums over n via ones-matmuls; rden = 1/sums ----
            pr = psR.tile([128, 192], F32, tag="R")
            for t in range(2):
                nc.tensor.matmul(
                    pr[0:1, 0:96], ones8[:],
                    pT[:].rearrange("p nt b h -> p nt (b h)")
                    [:, 2 * t:2 * t + 2, :],
                    start=(t == 0), stop=False, perf_mode=DR)
            nc.tensor.matmul(
                pr[0:1, 0:96], ones8[:NTAIL, 0, :],
                pT[:NTAIL, 4, :, :].rearrange("p b h -> p (b h)"),
                start=False, stop=True)
            nc.vector.reciprocal(rden[:], pr[0:1, 0:96])

            # ---- rdenB[o, (j h)] per group: broadcast rden down 128
            #      partitions with an outer-product matmul ----
            for g in range(2):
                nc.tensor.matmul(
                    pr[:, 96 + 48 * g:96 + 48 * (g + 1)],
                    onesf[:], rden[0:1, 48 * g:48 * (g + 1)],
                    start=True, stop=True)
            nc.vector.tensor_copy(
                rdenB[:].rearrange("p g u c -> p (g u c)"), pr[:, 96:192])

            # ---- ZT[c, (j h)] per (group, batch): 18 DR matmuls +
            #      normalize-and-cast evacuation ----
            for g in range(2):
                pz = psZ.tile([128, CT, 4, H], F32, tag="Z")
                for jj in range(4):
                    b = 4 * g + jj
                    x2 = x2s[b]
                    for ci in range(CT):
                        for t in range(2):
                            nc.tensor.matmul(
                                pz[:, ci, jj, :],
                                x2[:, 2 * t:2 * t + 2,
                                   128 * ci:128 * (ci + 1)],
                                pT[:, 2 * t:2 * t + 2, b, :],
                                start=(t == 0), stop=False, perf_mode=DR)
                        nc.tensor.matmul(
                            pz[:, ci, jj, :],
                            x2[:NTAIL, 4, 128 * ci:128 * (ci + 1)],
                            pT[:NTAIL, 4, b, :],
                            start=False, stop=True)
                nc.vector.tensor_mul(
                    ZT[:, :, g, :],
                    pz[:].rearrange("p a j h -> p a (j h)"),
                    rdenB[:, g, :, :].to_broadcast([128, CT, 48]))

                # ---- oT[o', b] for this group: head-diagonal blocks of
                #      wv.T @ ZT, computed directly (out free dim 4) ----
                if g == 0:
                    po = psA.tile([128, CT, BPC], F32, tag="A")
                ZTv = ZT[:].rearrange("p a g (j h) -> p a g j h", h=H)
                for ci in range(CT):
                    for hh in range(2):
                        h = 2 * ci + hh
                        base = 128 * ci + 64 * hh
                        for t in range(3):
                            nc.tensor.matmul(
                                po[64 * hh:64 * (hh + 1), ci, 4 * g:4 * (g + 1)],
                                wv[:, 2 * t:2 * t + 2, base:base + 64],
                                ZTv[:, 2 * t:2 * t + 2, g, :, h],
                                start=(t == 0), stop=(t == 2), perf_mode=DR,
                                tile_position=(0, 64 * hh))
                nc.vector.tensor_copy(
                    oT[:, :, 4 * g:4 * (g + 1)],
                    po[:, :, 4 * g:4 * (g + 1)])

                # ---- clsT[j, b] = wp.T @ oT + pbT, f32 out ----
                if g == 0:
                    pc = psA.tile([128, CT, BPC], F32, tag="A")
                for jc in range(CT):
                    for t in range(3):
                        nc.tensor.matmul(
                            pc[:, jc, 4 * g:4 * (g + 1)],
                            wp[:, 2 * t:2 * t + 2, 128 * jc:128 * (jc + 1)],
                            oT[:, 2 * t:2 * t + 2, 4 * g:4 * (g + 1)],
                            start=(t == 0), stop=(t == 2), perf_mode=DR)
                nc.vector.tensor_add(
                    clsT_sb[:, :, 4 * g:4 * (g + 1)],
                    pc[:, :, 4 * g:4 * (g + 1)],
                    pbT[:, :, 4 * g:4 * (g + 1)])
                nc.sync.dma_start(
                    clsT_d.ap()[:, 4 * g:4 * (g + 1)]
                    .rearrange("(a p) b -> p a b", p=128),
                    clsT_sb[:, :, 4 * g:4 * (g + 1)])

    nc.compile()
    return nc


@functools.lru_cache(maxsize=1)
def _module():
    return build_module()


def make_in_maps(x, qkv_w, qkv_b, proj_w, proj_b):
    x = np.asarray(x, dtype=np.float32)
    qkv_w = np.asarray(qkv_w, dtype=np.float32)
    qkv_b = np.asarray(qkv_b, dtype=np.float32)
    proj_w = np.asarray(proj_w, dtype=np.float32)
    proj_b = np.asarray(proj_b, dtype=np.float32)

    wq = np.ascontiguousarray(qkv_w[:C].T * SCALE).astype(NPBF16)   # [c, o]
    wk2 = np.ascontiguousarray(qkv_w[C:2 * C]).astype(NPBF16)       # [o, c]
    wv = np.ascontiguousarray(qkv_w[2 * C:].T).astype(NPF8)         # [c, o]
    wp = np.ascontiguousarray(proj_w.T).astype(NPF8)                # [c, o]
    # q-bias folds into Wt: wtqb[c, h] = wk_block_h[:, c] . qb_block_h
    qbs = qkv_b[:C] * SCALE
    wtqb1 = np.stack(
        [qkv_w[C + 64 * h:C + 64 * (h + 1)].T @ qbs[64 * h:64 * (h + 1)]
         for h in range(H)], axis=1)                                # [C, H]
    wtqb = np.tile(wtqb1, (1, BPC)).astype(NPBF16)                  # [C, 96]
    # v bias contributes exactly (vb @ proj_w.T) to cls; fold into proj bias
    pb_eff = proj_b + qkv_b[2 * C:] @ proj_w.T

    in_maps = []
    for i in range(NCORES):
        xs = x[i * BPC:(i + 1) * BPC]                               # [8, N, C]
        x2 = np.zeros((BPC * N + X2PAD, C), dtype=NPF8)
        x2[:BPC * N] = xs.reshape(BPC * N, C).astype(NPF8)
        xT = np.ascontiguousarray(xs.transpose(2, 0, 1)).astype(NPF8)
        xcls = np.ascontiguousarray(xs[:, 0, :].T).astype(NPBF16)   # [C, 8]
        pbT = np.tile(pb_eff[:, None], (1, BPC)).astype(np.float32)
        in_maps.append({
            "xT": xT, "x2": x2, "wq": wq, "wk2": wk2, "wv": wv, "wp": wp,
            "xcls": xcls, "wtqb": wtqb, "pbT": pbT,
        })
    return in_maps


def kernel(x, qkv_w, qkv_b, proj_w, proj_b):
    nc = _module()
    in_maps = make_in_maps(x, qkv_w, qkv_b, proj_w, proj_b)
    res = bass_utils.run_bass_kernel_spmd(
        nc, in_maps, core_ids=list(range(NCORES)))
    out = np.array(np.asarray(x), dtype=np.float32, copy=True)
    for i in range(NCORES):
        out[i * BPC:(i + 1) * BPC, 0, :] = res.results[i]["clsT"].T
    return out


# revision 17
# speedup vs baseline: 2.0267x; 1.0012x over previous
"""ClassAttention kernel for 8x TRN2 NeuronCores — fp8 DoubleRow rewrite.

Reference computation (per batch element):
    qkv = x @ qkv_w.T + qkv_b                      # [N, 3C]
    q, k, v = split(qkv)                           # heads H=12, D=64
    s = softmax((q_cls . k) / sqrt(D))             # class-token query only
    cls = (s @ v) @ proj_w.T + proj_b              # [1, C]
    out = concat([cls, x[1:]])                     # rows 1..N pass through

Only the class token row changes, so the device computes just the [B, C]
cls output (shipped transposed as clsT [C, B]); rows 1..N pass through on
the host.  Data-parallel over batch: 8 batches per core, no collectives.

Algebraic structure (inherited from the bf16 baseline):
  - k-projection folds into x-space:  s[b,h,n] = sum_c Wt[c,bh] x[b,n,c]
    with Wt = wk.T @ blockdiag(q) computed once on device; no k vector is
    materialized.  k-bias cancels in softmax; q-bias folds into Wt via a
    host-precomputed wtqb.
  - v-projection commutes with the attention average: the kernel averages
    x (ZT = x.T @ p) and projects through wv once; v-bias folds into the
    proj bias on the host.
  - softmax skips the max-shift (scores are O(1)); the 1/sum scaling is
    applied per (b,h) column during the ZT psum evacuation.

What is new vs the baseline (82.2us -> ~35us modeled):
  - fp8(e4m3) data path: x (both layouts), wv, wp, Wt, p=exp(s), ZT, oT
    are fp8; the score-weight path (wq, wk2, q, Qblk, Wt accumulation)
    stays bf16 because it dominates the error budget.  DoubleRow fp8
    matmuls (2 K-tiles per instruction, 0.5 cycles/row) carry all the
    heavy contractions.
  - every stage computes the TRANSPOSED output with a small moving free
    dim (qT, sT, ZT, oT, clsT), so there are ZERO data transposes and
    psum evacuations are few and wide ([128, .] copies, not [12, .]).
  - 21 large DMAs instead of 67 (HWDGE issue cost ~630ns each gated the
    baseline); x2 is read as [128, 5, 768] per batch from a 63-row-padded
    flat buffer so each batch is one descriptor-dense transfer.

Per-core dataflow (b = 0..8 batches, c in 6 chunks of 128):
  qT[o, b]        36 bf16 matmuls      (needs xcls, wq)
  Qblk[o, (b h)]  12 blockdiag copies  (DVE, psum -> bf16)
  Wt[c, (b h)]    36 bf16 matmuls + 6 adds (+wtqb, cast fp8)
  sT[n, (b h)]    120 DR matmuls       (needs all xT)
  pT = exp(sT-1)  2 Act ops, fp8       (bias cancels in the 1/sum)
  sums[1, (b h)]  3 ones-matmuls; rden = 1/sums (f32)
  rdenB[o, (b h)] 2 outer-product matmuls + copy
  ZT[c, (g j h)]  144 DR matmuls       (needs x2_b), x rden -> fp8
  oT[o', b]       72 DR matmuls        (diag blocks direct, needs wv)
  clsT[j, b]      36 DR matmuls + pbT add -> f32, DMA out per group
"""

import functools

import numpy as np
import ml_dtypes

import concourse.bass as bass
import concourse.tile as tile
from concourse import bacc, mybir
from concourse import bass_utils

BF16 = mybir.dt.bfloat16
F8 = mybir.dt.float8e4
F32 = mybir.dt.float32
NPBF16 = ml_dtypes.bfloat16
NPF8 = ml_dtypes.float8_e4m3
DR = mybir.MatmulPerfMode.DoubleRow

B, N, C = 64, 577, 768
H, D = 12, 64
NCORES = 8
BPC = B // NCORES          # 8 batches per core
CT = C // 128              # 6 chunks of the feature dim
NT = 5                     # token tiles of 128 (last holds 65)
NTAIL = N - 4 * 128        # 65
SCALE = D ** -0.5          # folded into wq on the host
X2PAD = 5 * 128 - N        # 63 rows of row padding after the last batch


def build_module():
    nc = bacc.Bacc("TRN2", target_bir_lowering=False, debug=False)

    xT_d = nc.dram_tensor("xT", [C, BPC, N], F8, kind="ExternalInput")
    x2_d = nc.dram_tensor("x2", [BPC * N + X2PAD, C], F8, kind="ExternalInput")
    wq_d = nc.dram_tensor("wq", [C, C], BF16, kind="ExternalInput")    # [c, o]
    wk2_d = nc.dram_tensor("wk2", [C, C], BF16, kind="ExternalInput")  # [o, c]
    wv_d = nc.dram_tensor("wv", [C, C], F8, kind="ExternalInput")      # [c, o]
    wp_d = nc.dram_tensor("wp", [C, C], F8, kind="ExternalInput")      # [c, o]
    xcls_d = nc.dram_tensor("xcls", [C, BPC], BF16, kind="ExternalInput")
    wtqb_d = nc.dram_tensor("wtqb", [C, BPC * H], BF16, kind="ExternalInput")
    pbT_d = nc.dram_tensor("pbT", [C, BPC], F32, kind="ExternalInput")
    clsT_d = nc.dram_tensor("clsT", [C, BPC], F32, kind="ExternalOutput")

    AF = mybir.ActivationFunctionType

    with tile.TileContext(nc) as tc:
        with (
            tc.tile_pool(name="sb", bufs=1) as sb,
            tc.tile_pool(name="psA", bufs=2, space="PSUM") as psA,
            tc.tile_pool(name="psW", bufs=2, space="PSUM") as psW,
            tc.tile_pool(name="psS", bufs=1, space="PSUM") as psS,
            tc.tile_pool(name="psR", bufs=1, space="PSUM") as psR,
            tc.tile_pool(name="psZ", bufs=2, space="PSUM") as psZ,
        ):
            # ---- DMAs, in consumption order (one channel, serialized) ----
            xcls = sb.tile([128, CT, BPC], BF16, tag="xcls")
            nc.sync.dma_start(
                xcls[:], xcls_d.ap().rearrange("(a p) b -> p a b", p=128))
            wq = sb.tile([128, CT, C], BF16, tag="wq")
            nc.sync.dma_start(
                wq[:], wq_d.ap().rearrange("(a p) o -> p a o", p=128))
            wk2 = sb.tile([128, CT, C], BF16, tag="wk2")
            nc.sync.dma_start(
                wk2[:], wk2_d.ap().rearrange("(a p) o -> p a o", p=128))
            wtqb = sb.tile([128, CT, BPC * H], BF16, tag="wtqb")
            nc.sync.dma_start(
                wtqb[:], wtqb_d.ap().rearrange("(a p) o -> p a o", p=128))
            # x in c-major layout, one DMA per batch; rows padded to 640 so
            # DoubleRow k-tile-pair slices have a 64-multiple stride (walrus
            # ISA requirement on Ldweights)
            xTs = []
            for b in range(BPC):
                xt = sb.tile([128, CT, 640], F8, tag=f"xT{b}")
                nc.sync.dma_start(
                    xt[:, :, 0:N],
                    xT_d.ap()[:, b, :].rearrange("(a p) t -> p a t", p=128))
                xTs.append(xt)
            wv = sb.tile([128, CT, C], F8, tag="wv")
            nc.sync.dma_start(
                wv[:], wv_d.ap().rearrange("(a p) o -> p a o", p=128))
            wp = sb.tile([128, CT, C], F8, tag="wp")
            nc.sync.dma_start(
                wp[:], wp_d.ap().rearrange("(a p) o -> p a o", p=128))
            pbT = sb.tile([128, CT, BPC], F32, tag="pbT")
            nc.sync.dma_start(
                pbT[:], pbT_d.ap().rearrange("(a p) b -> p a b", p=128))
            # x in token-major layout, one overlapping [640, C] read per
            # batch (rows past token 577 belong to the next batch / the host
            # pad and are masked by exact-K tail matmuls)
            x2s = []
            for b in range(BPC):
                x2 = sb.tile([128, NT, C], F8, tag=f"x2{b}")
                nc.sync.dma_start(
                    x2[:],
                    x2_d.ap()[b * N:b * N + NT * 128, :]
                    .rearrange("(a p) c -> p a c", p=128))
                x2s.append(x2)

            # ---- small constants ----
            ones8 = sb.tile([128, 2, 64], F8, tag="ones8")
            nc.vector.memset(ones8[:], 1.0)
            negone = sb.tile([128, 1], F32, tag="negone")
            nc.vector.memset(negone[:], -1.0)
            onesf = sb.tile([1, 128], F32, tag="onesf")
            nc.vector.memset(onesf[:], 1.0)
            Qblk = sb.tile([128, CT, BPC * H], BF16, tag="Qblk")
            nc.vector.memset(Qblk[:], 0.0)

            # fp8 operand tiles are padded so every DoubleRow k-pair slice
            # has a 64-multiple stride
            Wt = sb.tile([128, CT, 128], F8, tag="Wt")
            pT = sb.tile([128, NT, BPC, 16], F8, tag="pT")
            rden = sb.tile([1, BPC * H], F32, tag="rden")
            rdenB = sb.tile([128, 2, 1, 48], F32, tag="rdenB")
            ZT = sb.tile([128, CT, 2, 4, 16], F8, tag="ZT")
            oT = sb.tile([128, CT, 64], F8, tag="oT")
            clsT_sb = sb.tile([128, CT, BPC], F32, tag="clsT_sb")

            # ---- qT[o, b]: 36 bf16 matmuls, out free dim 8 ----
            pq = psA.tile([128, CT, BPC], F32, tag="A")
            for oc in range(CT):
                for ck in range(CT):
                    nc.tensor.matmul(
                        pq[:, oc, :],
                        wq[:, ck, 128 * oc:128 * (oc + 1)],
                        xcls[:, ck, :],
                        start=(ck == 0), stop=(ck == CT - 1))

            # ---- Qblk[o, (b h)]: blockdiag scatter of qT (bf16) ----
            QblkV = Qblk[:].rearrange("p a (b h) -> p a b h", h=H)
            for oc in range(CT):
                for j in range(2):
                    h = 2 * oc + j
                    nc.vector.tensor_copy(
                        QblkV[64 * j:64 * (j + 1), oc, :, h],
                        pq[64 * j:64 * (j + 1), oc, :])

            # ---- Wt[c, (b h)] = wk2.T @ Qblk + wtqb, cast fp8 ----
            for cj in range(CT):
                pw = psW.tile([128, BPC * H], F32, tag="W")
                for ok in range(CT):
                    nc.tensor.matmul(
                        pw[:], wk2[:, ok, 128 * cj:128 * (cj + 1)],
                        Qblk[:, ok, :],
                        start=(ok == 0), stop=(ok == CT - 1))
                nc.vector.tensor_add(Wt[:, cj, 0:BPC * H], pw[:], wtqb[:, cj, :])

            # ---- sT[n, (b h)] per batch: 15 DR matmuls over c ----
            ps_s = psS.tile([128, NT, BPC, H], F32, tag="S")
            for b in range(BPC):
                for nt in range(NT):
                    w = 128 if nt < NT - 1 else NTAIL
                    off = 128 * nt
                    for t in range(3):
                        nc.tensor.matmul(
                            ps_s[:w, nt, b, :],
                            xTs[b][:, 2 * t:2 * t + 2, off:off + w],
                            Wt[:, 2 * t:2 * t + 2, H * b:H * (b + 1)],
                            start=(t == 0), stop=(t == 2), perf_mode=DR)

            # ---- pT = exp(sT - 1), fp8 (the -1 cancels in 1/sum and
            #      keeps e below the fp8e4 max) ----
            nc.scalar.activation(
                pT[:, 0:4, :, 0:H], ps_s[:, 0:4, :, :], AF.Exp,
                bias=negone[:], scale=1.0)
            nc.scalar.activation(
                pT[:NTAIL, 4, :, 0:H], ps_s[:NTAIL, 4, :, :], AF.Exp,
                bias=negone[:NTAIL, :], scale=1.0)

            # ---- sums over n via ones-matmuls; rden = 1/sums ----
            pr = psR.tile([128, 192], F32, tag="R")
            for nt in range(NT):
                w = 128 if nt < NT - 1 else NTAIL
                nc.tensor.matmul(
                    pr[0:1, 0:96], ones8[:w, 0, 0:1],
                    pT[:w, nt, :, 0:H],
                    start=(nt == 0), stop=(nt == NT - 1))
            nc.vector.reciprocal(rden[:], pr[0:1, 0:96])

            # ---- rdenB[o, (j h)] per group: broadcast rden down 128
            #      partitions with an outer-product matmul ----
            for g in range(2):
                nc.tensor.matmul(
                    pr[:, 96 + 48 * g:96 + 48 * (g + 1)],
                    onesf[:], rden[0:1, 48 * g:48 * (g + 1)],
                    start=True, stop=True)
            nc.vector.tensor_copy(
                rdenB[:].rearrange("p g u c -> p (g u c)"), pr[:, 96:192])

            # ---- ZT[c, (j h)] per (group, batch): 18 DR matmuls +
            #      normalize-and-cast evacuation ----
            for g in range(2):
                pz = psZ.tile([128, CT, 4, H], F32, tag="Z")
                for jj in range(4):
                    b = 4 * g + jj
                    x2 = x2s[b]
                    for ci in range(CT):
                        for t in range(2):
                            nc.tensor.matmul(
                                pz[:, ci, jj, :],
                                x2[:, 2 * t:2 * t + 2,
                                   128 * ci:128 * (ci + 1)],
                                pT[:, 2 * t:2 * t + 2, b, 0:H],
                                start=(t == 0), stop=False, perf_mode=DR)
                        nc.tensor.matmul(
                            pz[:, ci, jj, :],
                            x2[:NTAIL, 4, 128 * ci:128 * (ci + 1)],
                            pT[:NTAIL, 4, b, 0:H],
                            start=False, stop=True)
                nc.vector.tensor_mul(
                    ZT[:, :, g, :, 0:H],
                    pz[:],
                    rdenB[:, g, :, :].rearrange("p u (j h) -> p u j h", h=H)
                    .to_broadcast([128, CT, 4, H]))

                # ---- oT[o', b] for this group: head-diagonal blocks of
                #      wv.T @ ZT, computed directly (out free dim 4) ----
                if g == 0:
                    po = psA.tile([128, CT, BPC], F32, tag="A")
                for ci in range(CT):
                    for hh in range(2):
                        h = 2 * ci + hh
                        base = 128 * ci + 64 * hh
                        for t in range(3):
                            nc.tensor.matmul(
                                po[64 * hh:64 * (hh + 1), ci, 4 * g:4 * (g + 1)],
                                wv[:, 2 * t:2 * t + 2, base:base + 64],
                                ZT[:, 2 * t:2 * t + 2, g, :, h],
                                start=(t == 0), stop=(t == 2), perf_mode=DR,
                                tile_position=(0, 64 * hh))
                nc.vector.tensor_copy(
                    oT[:, :, 4 * g:4 * (g + 1)],
                    po[:, :, 4 * g:4 * (g + 1)])

                # ---- clsT[j, b] = wp.T @ oT + pbT, f32 out ----
                if g == 0:
                    pc = psA.tile([128, CT, BPC], F32, tag="A")
                for jc in range(CT):
                    for t in range(3):
                        nc.tensor.matmul(
                            pc[:, jc, 4 * g:4 * (g + 1)],
                            wp[:, 2 * t:2 * t + 2, 128 * jc:128 * (jc + 1)],
                            oT[:, 2 * t:2 * t + 2, 4 * g:4 * (g + 1)],
                            start=(t == 0), stop=(t == 2), perf_mode=DR)
                nc.vector.tensor_add(
                    clsT_sb[:, :, 4 * g:4 * (g + 1)],
                    pc[:, :, 4 * g:4 * (g + 1)],
                    pbT[:, :, 4 * g:4 * (g + 1)])
                nc.sync.dma_start(
                    clsT_d.ap()[:, 4 * g:4 * (g + 1)]
                    .rearrange("(a p) b -> p a b", p=128),
                    clsT_sb[:, :, 4 * g:4 * (g + 1)])

    nc.compile()
    return nc


@functools.lru_cache(maxsize=1)
def _module():
    return build_module()


def make_in_maps(x, qkv_w, qkv_b, proj_w, proj_b):
    x = np.asarray(x, dtype=np.float32)
    qkv_w = np.asarray(qkv_w, dtype=np.float32)
    qkv_b = np.asarray(qkv_b, dtype=np.float32)
    proj_w = np.asarray(proj_w, dtype=np.float32)
    proj_b = np.asarray(proj_b, dtype=np.float32)

    wq = np.ascontiguousarray(qkv_w[:C].T * SCALE).astype(NPBF16)   # [c, o]
    wk2 = np.ascontiguousarray(qkv_w[C:2 * C]).astype(NPBF16)       # [o, c]
    wv = np.ascontiguousarray(qkv_w[2 * C:].T).astype(NPF8)         # [c, o]
    wp = np.ascontiguousarray(proj_w.T).astype(NPF8)                # [c, o]
    # q-bias folds into Wt: wtqb[c, h] = wk_block_h[:, c] . qb_block_h
    qbs = qkv_b[:C] * SCALE
    wtqb1 = np.stack(
        [qkv_w[C + 64 * h:C + 64 * (h + 1)].T @ qbs[64 * h:64 * (h + 1)]
         for h in range(H)], axis=1)                                # [C, H]
    wtqb = np.tile(wtqb1, (1, BPC)).astype(NPBF16)                  # [C, 96]
    # v bias contributes exactly (vb @ proj_w.T) to cls; fold into proj bias
    pb_eff = proj_b + qkv_b[2 * C:] @ proj_w.T

    in_maps = []
    for i in range(NCORES):
        xs = x[i * BPC:(i + 1) * BPC]                               # [8, N, C]
        x2 = np.zeros((BPC * N + X2PAD, C), dtype=NPF8)
        x2[:BPC * N] = xs.reshape(BPC * N, C).astype(NPF8)
        xT = np.ascontiguousarray(xs.transpose(2, 0, 1)).astype(NPF8)
        xcls = np.ascontiguousarray(xs[:, 0, :].T).astype(NPBF16)   # [C, 8]
        pbT = np.tile(pb_eff[:, None], (1, BPC)).astype(np.float32)
        in_maps.append({
            "xT": xT, "x2": x2, "wq": wq, "wk2": wk2, "wv": wv, "wp": wp,
            "xcls": xcls, "wtqb": wtqb, "pbT": pbT,
        })
    return in_maps


def kernel(x, qkv_w, qkv_b, proj_w, proj_b):
    nc = _module()
    in_maps = make_in_maps(x, qkv_w, qkv_b, proj_w, proj_b)
    res = bass_utils.run_bass_kernel_spmd(
        nc, in_maps, core_ids=list(range(NCORES)))
    out = np.array(np.asarray(x), dtype=np.float32, copy=True)
    for i in range(NCORES):
        out[i * BPC:(i + 1) * BPC, 0, :] = res.results[i]["clsT"].T
    return out
